# revision 12
# baseline (speedup 1.0000x reference)
"""Trainium2 Bass kernel for a bidirectional LSTM encoder head.

The model: h = tanh(E[tokens] @ W0 + b0); y_fw/y_bw = bidirectional
length-masked LSTM (relu activation, TF gate order i,g,f,o, forget bias
+1.0); output = concat([y_fw[-1], y_bw[-1]], axis=1) @ P.

Only the LAST batch element reaches the output, so we scan one sequence
per direction: core 0 forward, core 1 backward (same program, different
weights/token order).  The scan runs L = lengths[-1] steps.

Scan structure (the optimization): per step the recurrent matvec
h[384] @ Wh[384,1536] runs as 36 (ldweights, matmul N=1) pairs on PE.
Gate columns are grouped by OUTPUT hidden chunk (3 chunks of 128), gate
order [f, i, o, g].  The f/i/o groups for all chunks are computed first;
their sigmoids (one ACT instr per chunk) and everything else off the
critical path run under the PE weight stream.  The g groups come last,
and the post-matmul critical path is 3 small DVE ops per chunk:

    u  = sig_i * relu(pg_g + xp_g)        (IG2: fused xp add via s0)
    c' = sig_f * c + u                    (CU:  c via per-partition s0)
    h  = relu(c' * sig_o) -> ysT[:,c,t]   (H)

xp (= hs @ Wx + bias, incl. forget bias) is precomputed for all t.
"""

import sys

sys.path.insert(0, "/opt/trn_rl_repo")

from contextlib import ExitStack

import ml_dtypes
import numpy as np

import concourse.bacc as bacc
import concourse.bass as bass
import concourse.mybir as mybir
import concourse.tile as tile
from concourse.bass_utils import run_bass_kernel_spmd
from concourse.masks import make_identity

F32 = mybir.dt.float32
BF16 = mybir.dt.bfloat16
I32 = mybir.dt.int32

B, T, V, NE, NF, NR, NC = 128, 512, 50000, 300, 300, 300, 64
HPAD = 384  # padded hidden (3 chunks of 128)
GPAD = 1536  # padded gates (12 chunks of 128)
KC = 3  # hidden/embedding contraction chunks
GC = 12  # gate column chunks
SIG = mybir.ActivationFunctionType.Sigmoid
TANH = mybir.ActivationFunctionType.Tanh

# within-chunk gate order (column j = chunk*4 + index below)
GF, GI, GO, GG = 0, 1, 2, 3


def _register_fused_ops():
    """Register the LSTM cell fusions as custom DVE ops (per-NEFF table).

    IG2: out = in0 * relu(in1 + s0)        (sig_i * relu(g_pre + xp_g))
    CU : out = in0 * s0 + in1              (sig_f * c_prev + u)
    H  : out = relu(in0 * in1)             (relu(c_new) * sig_o; sig_o > 0)
    """
    import numpy as _np

    from concourse.dve_ops import (
        OPS,
        DveOp,
        DveOpSpec,
        get_dve_sub_opcode,
        has_src1,
    )
    from concourse.dve_spec import C0, Spec, Src0, Src1, lower, relu

    from concourse import dve_ops as _d

    if any(op.name == "ANT_LSTM_IG2" for op in OPS):
        return _d.ANT_LSTM_IG2, _d.ANT_LSTM_CU, _d.ANT_LSTM_H2  # type: ignore[attr-defined]

    defs = [
        ("ANT_LSTM_IG2", Spec(
            body=Src0 * relu(Src1 + C0),
            reference=lambda in0, in1, s0: in0 * _np.maximum(in1 + s0, 0))),
        ("ANT_LSTM_CU", Spec(
            body=Src0 * C0 + Src1,
            reference=lambda in0, in1, s0: in0 * s0 + in1)),
        ("ANT_LSTM_H2", Spec(
            body=relu(Src0 * Src1),
            reference=lambda in0, in1: _np.maximum(in0 * in1, 0))),
    ]
    made = []
    for name, spec in defs:
        op = DveOp(name, spec, subdim=False, uops_sha={})
        OPS.append(op)
        _d._SUB_OPCODE_FOR_NAME[name] = _d._CUSTOM_DVE_ROW_BASE + len(OPS) - 1
        _d.CUSTOM_DVE_SPECS[name] = spec
        for ver in ("v3", "v4"):
            r = DveOpSpec(
                name=name,
                opcode=get_dve_sub_opcode(name),
                uops=lower(spec, ver=ver),
                rd1_en=has_src1(spec),
            )
            op.uops_sha[ver] = r.sha(ver)
        made.append(op)
    _d.ANT_LSTM_IG2, _d.ANT_LSTM_CU, _d.ANT_LSTM_H2 = made  # type: ignore[attr-defined]
    return made[0], made[1], made[2]


def build_program(L: int, scan_repeats: int = 1) -> bass.Bass:
    nc = bacc.Bacc()

    tok_d = nc.dram_tensor("tok4", [B, 4], I32, kind="ExternalInput")
    e_d = nc.dram_tensor("emb_table", [V, NE], F32, kind="ExternalInput")
    w0_d = nc.dram_tensor("w0t", [128, KC, HPAD], F32, kind="ExternalInput")
    b0_d = nc.dram_tensor("b0t", [128, KC], F32, kind="ExternalInput")
    wx_d = nc.dram_tensor("wxt", [128, KC, GPAD], F32, kind="ExternalInput")
    bias_d = nc.dram_tensor("biast", [128, GC], F32, kind="ExternalInput")
    wh_d = nc.dram_tensor("wht", [128, KC, GPAD], BF16, kind="ExternalInput")
    pp_d = nc.dram_tensor("ppt", [128, KC, NC], BF16, kind="ExternalInput")
    out_d = nc.dram_tensor("out", [NC, T], F32, kind="ExternalOutput")

    OP_IG2, OP_CU, OP_H = _register_fused_ops()

    with ExitStack() as ctx:
        tc = ctx.enter_context(tile.TileContext(nc))
        const = ctx.enter_context(tc.tile_pool(name="const", bufs=1))
        work = ctx.enter_context(tc.tile_pool(name="work", bufs=6))

        # ---- persistent SBUF tensors -------------------------------------
        w0_sb = const.tile([128, KC, HPAD], F32, tag="w0")
        wx_sb = const.tile([128, KC, GPAD], F32, tag="wx")
        wh_sb = const.tile([128, KC, GPAD], BF16, tag="wh")
        pp_sb = const.tile([128, KC, NC], BF16, tag="pp")
        b0_sb = const.tile([128, KC], F32, tag="b0")
        bias_sb = const.tile([128, GC], F32, tag="bias")
        tok_sb = const.tile([128, 4], I32, tag="tok")
        ident = const.tile([128, 128], F32, tag="ident")
        emb_sb = [
            const.tile([128, NE], F32, tag=f"emb{i}", name=f"emb{i}") for i in range(4)
        ]
        embT = const.tile([128, KC, T], F32, tag="embT")
        hsT = const.tile([128, KC, T], F32, tag="hsT")
        xp = const.tile([128, GC, T], F32, tag="xp")
        ysT = const.tile([128, KC, T], BF16, tag="ysT")
        c_sb = const.tile([128, 2, KC], F32, tag="c")  # c state, parity-buffered
        z_sb = const.tile([128, T], F32, tag="z")

        nc.sync.dma_start(out=w0_sb[:], in_=w0_d[:])
        nc.sync.dma_start(out=wx_sb[:], in_=wx_d[:])
        nc.sync.dma_start(out=wh_sb[:], in_=wh_d[:])
        nc.sync.dma_start(out=pp_sb[:], in_=pp_d[:])
        nc.sync.dma_start(out=b0_sb[:], in_=b0_d[:])
        nc.sync.dma_start(out=bias_sb[:], in_=bias_d[:])
        nc.sync.dma_start(out=tok_sb[:], in_=tok_d[:])
        make_identity(nc, ident[:])

        # zero-init: embT (pad lanes must not be NaN), ysT (t>=L and pads)
        nc.vector.memset(embT[:], 0.0)
        nc.vector.memset(ysT[:], 0.0)
        nc.vector.memset(c_sb[:], 0.0)

        # ---- embedding gather (rows, scan order) -> transpose ------------
        for i in range(4):
            nc.gpsimd.indirect_dma_start(
                out=emb_sb[i][:],
                out_offset=None,
                in_=e_d[:],
                in_offset=bass.IndirectOffsetOnAxis(ap=tok_sb[:, i : i + 1], axis=0),
            )

        with tc.tile_pool(name="tp", bufs=2, space="PSUM") as tp_pool:
            for i in range(4):
                for c in range(KC):
                    w = min(NE, 128 * (c + 1)) - 128 * c  # 128,128,44
                    tp = tp_pool.tile([128, 128], F32, tag="tp")
                    nc.tensor.transpose(
                        out=tp[:w, :],
                        in_=emb_sb[i][:, 128 * c : 128 * c + w],
                        identity=ident[:],
                    )
                    nc.vector.tensor_copy(
                        out=embT[:w, c, 128 * i : 128 * (i + 1)], in_=tp[:w, :]
                    )

        # ---- h = tanh(emb @ W0 + b0), transposed layout ------------------
        with tc.tile_pool(name="mm", bufs=2, space="PSUM") as mm_pool:
            for m in range(KC):
                ph = mm_pool.tile([128, T], F32, tag="ph")
                for c in range(KC):
                    nc.tensor.matmul(
                        ph[:, :L],
                        lhsT=w0_sb[:, c, 128 * m : 128 * (m + 1)],
                        rhs=embT[:, c, :L],
                        start=(c == 0),
                        stop=(c == KC - 1),
                    )
                nc.scalar.activation(
                    out=hsT[:, m, :L], in_=ph[:, :L], func=TANH,
                    bias=b0_sb[:, m : m + 1],
                )

            # ---- xpart = hs @ Wx + bias (includes forget bias) -----------
            for j in range(GC):
                px = mm_pool.tile([128, T], F32, tag="ph")
                for c in range(KC):
                    nc.tensor.matmul(
                        px[:, :L],
                        lhsT=wx_sb[:, c, 128 * j : 128 * (j + 1)],
                        rhs=hsT[:, c, :L],
                        start=(c == 0),
                        stop=(c == KC - 1),
                    )
                nc.vector.tensor_scalar_add(
                    out=xp[:, j, :L], in0=px[:, :L], scalar1=bias_sb[:, j : j + 1]
                )

        # ---- the scan ----------------------------------------------------
        # Each chunk's f/i/o gate preacts and the g preacts live in SEPARATE
        # PSUM tiles (= separate banks): the DVE adds read chunk co's oif
        # tile while PE is still writing other chunks' columns; same-bank
        # adjacency would serialize PE behind the cell ops.
        pg_pool = ctx.enter_context(tc.tile_pool(name="pg", bufs=1, space="PSUM"))

        def cell_tail(co, s, u_src, u_s0, t):
            """Post-g critical path for chunk co at step t: IG2 -> CU -> H."""
            u = work.tile([128, 1], F32, tag="u")
            nc.vector._custom_dve(
                OP_IG2, out=u[:], in0=s[:, GI : GI + 1], in1=u_src, s0=u_s0
            )
            nc.vector._custom_dve(
                OP_CU,
                out=c_sb[:, t % 2, co : co + 1],
                in0=s[:, GF : GF + 1],
                in1=u[:],
                s0=c_sb[:, (t - 1) % 2, co : co + 1] if t > 0 else 0.0,
            )
            nc.vector._custom_dve(
                OP_H,
                out=ysT[:, co, t : t + 1],
                in0=c_sb[:, t % 2, co : co + 1],
                in1=s[:, GO : GO + 1],
            )

        for _rep in range(scan_repeats):
            # t = 0: gates are xp alone (h_{-1} = 0, c_{-1} = 0)
            s0tiles = []
            for co in range(KC):
                s = work.tile([128, 3], F32, tag="s")
                nc.scalar.activation(
                    out=s[:], in_=xp[:, 4 * co : 4 * co + 3, 0], func=SIG
                )
                s0tiles.append(s)
            for co in range(KC):
                cell_tail(co, s0tiles[co], xp[:, 4 * co + GG, 0:1], 0.0, 0)

            for t in range(1, L):
                pgs = [
                    pg_pool.tile([128, 3], F32, tag=f"pg_oif{co}",
                                 name=f"pg_oif{co}")
                    for co in range(KC)
                ]
                pgg = pg_pool.tile([128, KC], F32, tag="pg_g")
                stiles = []
                # f/i/o gate groups, chunk-major; sig per chunk under PE
                for co in range(KC):
                    for k, gi in enumerate((GF, GI, GO)):
                        col = 4 * co + gi
                        for c in range(KC):
                            nc.tensor.matmul(
                                pgs[co][:, k : k + 1],
                                lhsT=wh_sb[:, c, 128 * col : 128 * (col + 1)],
                                rhs=ysT[:, c, t - 1 : t],
                                start=(c == 0),
                                stop=(c == KC - 1),
                            )
                    g0 = work.tile([128, 3], F32, tag="g0")
                    nc.vector.tensor_add(
                        out=g0[:],
                        in0=pgs[co][:],
                        in1=xp[:, 4 * co : 4 * co + 3, t],
                    )
                    s = work.tile([128, 3], F32, tag="s")
                    nc.scalar.activation(out=s[:], in_=g0[:], func=SIG)
                    stiles.append(s)
                # g gate groups (separate PSUM bank), then the 3-op tail
                for co in range(KC):
                    col = 4 * co + GG
                    for c in range(KC):
                        nc.tensor.matmul(
                            pgg[:, co : co + 1],
                            lhsT=wh_sb[:, c, 128 * col : 128 * (col + 1)],
                            rhs=ysT[:, c, t - 1 : t],
                            start=(c == 0),
                            stop=(c == KC - 1),
                        )
                for co in range(KC):
                    cell_tail(
                        co,
                        stiles[co],
                        pgg[:, co : co + 1],
                        xp[:, 4 * co + GG, t : t + 1],
                        t,
                    )

        # ---- z^T = P_half^T @ ys^T  -> [64, T] ---------------------------
        with tc.tile_pool(name="zp", bufs=1, space="PSUM") as zp_pool:
            pz = zp_pool.tile([128, T], F32, tag="pz")
            for c in range(KC):
                nc.tensor.matmul(
                    pz[:NC, :],
                    lhsT=pp_sb[:, c, :],
                    rhs=ysT[:, c, :],
                    start=(c == 0),
                    stop=(c == KC - 1),
                )
            nc.vector.tensor_copy(out=z_sb[:NC, :], in_=pz[:NC, :])
        nc.sync.dma_start(out=out_d[:], in_=z_sb[:NC, :])

    nc.compile()
    return nc


def _prep_gate_weights(W, b):
    """W: [600, 1200] (rows 0:300 x-part, 300:600 h-part), cols in TF order
    i,g,f,o.  Returns Wx_pad [384,1536], Wh_pad [384,1536], bias_pad [1536]
    in our layout: column block j = 4*chunk + {f,i,o,g}, each block 128 gate
    units = units [128*chunk : 128*chunk+128) of that gate (zero-padded past
    300); +1.0 forget bias on f."""
    secs = {GF: 600, GI: 0, GO: 900, GG: 300}  # TF col offsets: i,g,f,o
    Wx = np.zeros((HPAD, GPAD), np.float32)
    Wh = np.zeros((HPAD, GPAD), np.float32)
    bias = np.zeros((GPAD,), np.float32)
    for co in range(KC):
        lo = 128 * co
        w = min(300, lo + 128) - lo  # 128,128,44
        for gi in range(4):
            col = (4 * co + gi) * 128
            src = secs[gi] + lo
            Wx[:NF, col : col + w] = W[:NF, src : src + w]
            Wh[:NR, col : col + w] = W[NF : NF + NR, src : src + w]
            bias[col : col + w] = b[src : src + w]
            if gi == GF:
                bias[col : col + w] += 1.0  # TF BasicLSTMCell forget bias
    return Wx, Wh, bias


def _core_inputs(tokens_ord, E, W0, b0, W, bgate, P_half):
    Wx, Wh, bias = _prep_gate_weights(np.asarray(W, np.float32), np.asarray(bgate))
    W0p = np.zeros((HPAD, HPAD), np.float32)
    W0p[:NE, :NF] = np.asarray(W0, np.float32)
    b0p = np.zeros((HPAD,), np.float32)
    b0p[:NF] = np.asarray(b0, np.float32).reshape(-1)
    Pp = np.zeros((HPAD, NC), np.float32)
    Pp[:NR] = np.asarray(P_half, np.float32)

    def chunked(M, width):  # [384, width] -> [128, KC, width]
        return np.ascontiguousarray(M.reshape(KC, 128, width).transpose(1, 0, 2))

    return {
        "tok4": np.ascontiguousarray(
            np.asarray(tokens_ord, np.int32).reshape(4, 128).T
        ),
        "emb_table": np.ascontiguousarray(np.asarray(E, np.float32)),
        "w0t": chunked(W0p, HPAD),
        "b0t": np.ascontiguousarray(b0p.reshape(KC, 128).T),
        "wxt": chunked(Wx, GPAD),
        "biast": np.ascontiguousarray(bias.reshape(GC, 128).T),
        "wht": chunked(Wh, GPAD).astype(ml_dtypes.bfloat16),
        "ppt": chunked(Pp, NC).astype(ml_dtypes.bfloat16),
    }


def _run(tokens, lengths, E, W0, b0, Wf, bf, Wb, bb, P, trace=False):
    tokens = np.asarray(tokens)
    lengths = np.asarray(lengths)
    L = int(lengths[B - 1])
    t_ar = np.arange(T)
    pos_bw = np.where(t_ar < L, L - 1 - t_ar, t_ar)

    tok_last = np.asarray(tokens[B - 1], np.int32)
    in_fw = _core_inputs(tok_last, E, W0, b0, Wf, bf, P[:NR])
    in_bw = _core_inputs(tok_last[pos_bw], E, W0, b0, Wb, bb, P[NR:])

    nc = build_program(L)
    n_cores = 8
    in_maps = [in_fw, in_bw] + [in_fw] * (n_cores - 2)
    res = run_bass_kernel_spmd(nc, in_maps, list(range(n_cores)), trace=trace)

    z_fw = np.asarray(res.results[0]["out"], np.float32).T  # [T, 64]
    z_bw = np.asarray(res.results[1]["out"], np.float32).T
    out = z_fw + z_bw[pos_bw]
    return out.astype(np.float32), res


def kernel(tokens, lengths, E, W0, b0, Wf, bf, Wb, bb, P):
    out, _ = _run(tokens, lengths, E, W0, b0, Wf, bf, Wb, bb, P)
    return out
